# revision 3
# baseline (speedup 1.0000x reference)
"""Trainium2 Bass kernel for nn_BinaryMemory (retrieval_knn) — v2.

reference:
    gated = sigmoid(query @ W.T + b)                      # [1, D], D=4096
    sims  = 1 - mean(|memory - gated|, axis=-1)           # [N],   N=16384
    mask  = sims >= 0.8

Sharding (8 cores, no collectives): shard the D axis; core c owns dims
[c*512, (c+1)*512).

v2 layout: d on PARTITIONS (transposed memory), fp8 everywhere.
  - gate: z = q @ W.T via fp8 DoubleRow matmuls (16 accumulating pairs),
    + b, sigmoid on ScalarE, then PE-transpose g into per-partition
    columns g_cols[128, 4] (one per d-subtile).
  - elementwise work split across THREE engines (the tensor_scalar ISA
    has no abs op, so DVE/GpSimd produce min while ScalarE produces abs):
      DVE:    diff = min(m, g)      tensor_scalar(op0=min)
      GpSimd: diff = min(m, g)      tensor_scalar(op0=min)
      Scalar: diff = |m - g|        activation(Abs, scale=-1, bias=g)
  - d-reduction on PE with SIGNED selector stationaries (fp8 DoubleRow):
    identity sum_d |m-g| = sum_abs |m-g| + sum_min (m + g - 2*min):
      R1: sel1 (col c: +1 abs-slices, -2 min-slices) over diff tiles
      R2: sel2 (col c: +1 min-slices only)           over RAW m tiles
      G:  selG (dense w2)                            over g8 columns
    R1+R2 accumulate into one psum [32, 512]; G into psum [32, 1];
    partial[c, n] = R1+R2 + G[c]  (added during evacuation).
  - evac: tensor_scalar_add(psum, G) -> sbuf f16 -> DMA [32, 512] out.

Per-core HBM: mem 8MB + W 2MB fp8 + sel ~1MB ~= 11.2MB.
"""
import sys

sys.path.insert(0, "/opt/trn_rl_repo")

import ml_dtypes
import numpy as np

import concourse.bacc as bacc
import concourse.mybir as mybir
import concourse.tile as tile
from concourse.bass_utils import run_bass_kernel_spmd

N_CORES = 8
D = 4096
N = 16384
D_SH = D // N_CORES          # 512 dims per core
PAIRS = 2                    # d-subtile DoubleRow pairs per core
W_PAIRS = D // 256           # 16 gate contraction pairs
NBLK = 4096                  # n per mem DMA block
NB = N // NBLK               # 8 blocks
NCH = 512                    # psum-bank n-chunk
THRESHOLD = 0.8

F8 = mybir.dt.float8e4
NP_F8 = ml_dtypes.float8_e4m3
F16_BLOCKS = ((2, 0),)   # (nb, pair) mem blocks shipped fp16 -> DVE 2x

_CACHE = {}


def _assign_slices():
    """Greedy-balance the 32 elementwise slices [128, 2048] across
    DVE(min) / Scalar(abs) / GpSimd(min) by modeled cost (ns/slice).
    Returns dict (nb, pair, s, half) -> 'v'|'s'|'g'."""
    cost = {"v": 2205.0, "s": 2001.0}
    load = {"v": len(F16_BLOCKS) * 2 * 2270.0 + 1400.0, "s": 0.0}
    out = {}
    for nb in range(NB):
        for pair in range(PAIRS):
            for s in range(2):
                for h in range(2):
                    if (nb, pair) in F16_BLOCKS:
                        out[(nb, pair, s, h)] = "v"
                        continue
                    e = min(cost, key=lambda k: load[k] + cost[k])
                    load[e] += cost[e]
                    out[(nb, pair, s, h)] = e
    return out


SLICE_ENG = _assign_slices()


def _r2_needed(nb, pair, c):
    cpb = NBLK // NCH
    h = (c % cpb) // (cpb // 2)
    return any(
        SLICE_ENG[(nb, pair, s, h)] != "s" for s in range(2)
    )


def _slice_is_min(nb, pair, s, h):
    return SLICE_ENG[(nb, pair, s, h)] != "s"


def _build():
    f32 = mybir.dt.float32
    f16 = mybir.dt.float16
    DR = mybir.MatmulPerfMode.DoubleRow
    nc = bacc.Bacc(
        "TRN2", target_bir_lowering=False, debug=False, num_devices=N_CORES
    )

    qd = nc.dram_tensor("qd", [128, 2 * W_PAIRS], F8, kind="ExternalInput")
    wd = nc.dram_tensor("wd", [128, W_PAIRS * 2 * D_SH], F8, kind="ExternalInput")
    bd = nc.dram_tensor("bd", [128, 4], f32, kind="ExternalInput")
    oned = nc.dram_tensor("oned", [1, 128], f32, kind="ExternalInput")
    sel1d = nc.dram_tensor("sel1d", [128, PAIRS * 32 * 2 * 32], F8, kind="ExternalInput")
    sel2d = nc.dram_tensor("sel2d", [128, PAIRS * 32 * 2 * 32], F8, kind="ExternalInput")
    selgd = nc.dram_tensor("selgd", [128, PAIRS * 2 * 32], F8, kind="ExternalInput")
    memd = nc.dram_tensor("memd", [PAIRS, 128, 2, N], F8, kind="ExternalInput")
    memd16 = nc.dram_tensor(
        "memd16", [max(1, len(F16_BLOCKS)), 128, 2, NBLK], f16,
        kind="ExternalInput",
    )
    partials = nc.dram_tensor("partials", [32, NCH], f16, kind="ExternalOutput")

    with tile.TileContext(nc) as tc:
        with (
            tc.tile_pool(name="const", bufs=1) as cpool,
            tc.tile_pool(name="mem", bufs=6) as mpool,
            tc.tile_pool(name="mem16", bufs=2) as m16pool,
            tc.tile_pool(name="diff", bufs=4) as dpool,
            tc.tile_pool(name="diff16", bufs=2) as d16pool,
            tc.tile_pool(name="small", bufs=1) as spool,
            tc.tile_pool(name="psum", bufs=1, space="PSUM") as ppool,
        ):
            # PE warmup: ramp the p-state before the gate matmuls arrive
            wrm = spool.tile([128, 1024], F8, tag="wrm")
            nc.vector.memset(wrm[:], 0.0)
            wrm_ps = ppool.tile([1, D_SH], f32, tag="wrmps")
            wv_warm = wrm[:].rearrange("p (s n) -> p s n", s=2)
            for i in range(8):
                nc.tensor.matmul(
                    wrm_ps[:],
                    wv_warm[:, :, 0:1],
                    wv_warm[:, :, 0:512],
                    start=(i == 0),
                    stop=(i == 7),
                    perf_mode=DR,
                )

            # scalar ring: q/b/one first (gate inputs), then selectors
            q_sb = cpool.tile([128, 2 * W_PAIRS], F8, tag="q")
            nc.scalar.dma_start(out=q_sb[:], in_=qd[:])
            bt_sb = spool.tile([128, 4], f32, tag="bt")
            nc.scalar.dma_start(out=bt_sb[:], in_=bd[:])
            one_sb = spool.tile([1, 128], f32, tag="one")
            nc.scalar.dma_start(out=one_sb[:], in_=oned[:])
            sel1_sb = cpool.tile([128, PAIRS * 32 * 2 * 32], F8, tag="sel1")
            nc.scalar.dma_start(out=sel1_sb[:], in_=sel1d[:])
            sel2_sb = cpool.tile([128, PAIRS * 32 * 2 * 32], F8, tag="sel2")
            nc.scalar.dma_start(out=sel2_sb[:], in_=sel2d[:])
            selg_sb = cpool.tile([128, PAIRS * 2 * 32], F8, tag="selg")
            nc.scalar.dma_start(out=selg_sb[:], in_=selgd[:])
            # bulk sync ring: W in 4 chunks (gate consumes per-chunk), then mem
            w_sb = cpool.tile([128, W_PAIRS * 2 * D_SH], F8, tag="w")
            WCH = W_PAIRS // 2
            for wc in range(2):
                lo = wc * WCH * 2 * D_SH
                hi = (wc + 1) * WCH * 2 * D_SH
                nc.sync.dma_start(out=w_sb[:, lo:hi], in_=wd[:][:, lo:hi])

            # ---- gate: z[l] = sum_d W[base+l, d] q[d] ----
            z_ps = ppool.tile([1, D_SH], f32, tag="zps")
            qv = q_sb[:].rearrange("p (s t) -> p s t", s=2)
            wv = w_sb[:].rearrange("p (t s l) -> p t s l", t=W_PAIRS, s=2)
            for t in range(W_PAIRS):
                nc.tensor.matmul(
                    z_ps[:],
                    qv[:, :, t : t + 1],
                    wv[:, t],
                    start=(t == 0),
                    stop=(t == W_PAIRS - 1),
                    perf_mode=DR,
                )
            # transpose z FIRST (tiny), then +b / sigmoid in column layout
            z_sb = spool.tile([1, D_SH], f16, tag="zsb")
            nc.vector.tensor_copy(z_sb[:], z_ps[:])
            one16 = spool.tile([1, 1], f16, tag="one16")
            nc.vector.tensor_copy(one16[:], one_sb[0:1, 0:1])
            zt_ps = ppool.tile([128, 4], f32, tag="ztps")
            for t in range(4):
                nc.tensor.matmul(
                    zt_ps[:, t : t + 1],
                    z_sb[0:1, t * 128 : (t + 1) * 128],
                    one16[:],
                    start=True,
                    stop=True,
                )
            zbt = spool.tile([128, 4], f32, tag="zbt")
            nc.vector.tensor_add(
                zbt[:], zt_ps[:], bt_sb[:]
            )
            g_cols = spool.tile([128, 4], f32, tag="gcols")
            nc.scalar.activation(
                g_cols[:], zbt[:], mybir.ActivationFunctionType.Sigmoid
            )
            g8rep = spool.tile([128, 4 * 128], F8, tag="g8rep")
            nc.vector.tensor_copy(
                g8rep[:].rearrange("p (t n) -> p t n", t=4),
                g_cols[:, :, None].broadcast_to([128, 4, 128]),
            )
            g8_cols = spool.tile([128, 4], F8, tag="g8cols")
            nc.vector.tensor_copy(g8_cols[:], g_cols[:])
            if F16_BLOCKS:
                g16rep = spool.tile([128, 4 * 128], f16, tag="g16rep")
                nc.vector.tensor_copy(
                    g16rep[:].rearrange("p (t n) -> p t n", t=4),
                    g_cols[:, :, None].broadcast_to([128, 4, 128]),
                )

            # ---- G[c] = sum_{min-slices} g  (per-chunk constant) ----
            gacc_ps = ppool.tile([32, 1], f32, tag="gacc")
            selgv = selg_sb[:].rearrange("p (a s m) -> p a s m", a=PAIRS, s=2)
            g8v = g8_cols[:].rearrange("p (a s) -> p a s", a=PAIRS)
            for pair in range(PAIRS):
                nc.tensor.matmul(
                    gacc_ps[:],
                    selgv[:, pair],
                    g8v[:, pair].rearrange("p (s o) -> p s o", o=1),
                    start=(pair == 0),
                    stop=(pair == PAIRS - 1),
                    perf_mode=DR,
                )
            g_col_sb = spool.tile([32, 1], f32, tag="gcol32")
            nc.vector.tensor_copy(g_col_sb[:], gacc_ps[:])

            # ---- mem stream: R2 (raw m), elementwise, R1 (diff) ----
            acc_lo = ppool.tile([32, NCH], f32, tag="acclo")
            acc_hi = ppool.tile([32, NCH], f32, tag="acchi")
            sel1v = sel1_sb[:].rearrange(
                "p (a c s m) -> p a c s m", a=PAIRS, c=32, s=2
            )
            sel2v = sel2_sb[:].rearrange(
                "p (a c s m) -> p a c s m", a=PAIRS, c=32, s=2
            )
            n_mm = {0: 0, 1: 0}
            _tot = {0: 0, 1: 0}
            for _nb in range(NB):
                for _pair in range(PAIRS):
                    for _c4 in range(NBLK // NCH):
                        _c = _nb * (NBLK // NCH) + _c4
                        _half = _c // 16
                        if (_nb, _pair) in F16_BLOCKS:
                            _tot[_half] += 4
                            continue
                        _tot[_half] += 1
                        if _r2_needed(_nb, _pair, _c):
                            _tot[_half] += 1
            n_mm_last = {h: _tot[h] - 1 for h in (0, 1)}

            def acc_mm(c, lhsT, rhs, dr):
                h = c // 16
                tile_ = acc_lo if h == 0 else acc_hi
                nc.tensor.matmul(
                    tile_[:],
                    lhsT,
                    rhs,
                    start=(n_mm[h] == 0),
                    stop=(n_mm[h] == n_mm_last[h]),
                    perf_mode=(
                        mybir.MatmulPerfMode.DoubleRow if dr else None
                    ),
                )
                n_mm[h] += 1
            f16_idx = {blk: i for i, blk in enumerate(F16_BLOCKS)}
            out_lo = spool.tile([32, NCH], f16, tag="outlo")
            for nb in range(NB):
                for pair in range(PAIRS):
                    if (nb, pair) in f16_idx:
                        bi = f16_idx[(nb, pair)]
                        m16 = m16pool.tile([128, 2 * NBLK], f16, tag="m16")
                        nc.sync.dma_start(
                            out=m16[:].rearrange("p (s n) -> p s n", s=2),
                            in_=memd16[:][bi],
                        )
                        d16 = d16pool.tile([128, 2 * NBLK], f16, tag="d16")
                        reps = NBLK // 128
                        for s in range(2):
                            tt = 2 * pair + s
                            g16r = g16rep[
                                :, tt * 128 : (tt + 1) * 128
                            ][:, None, :].broadcast_to([128, reps, 128])
                            lo = s * NBLK
                            hi = lo + NBLK
                            nc.vector.tensor_tensor(
                                d16[:, lo:hi].rearrange(
                                    "p (o n) -> p o n", o=reps
                                ),
                                m16[:, lo:hi].rearrange(
                                    "p (o n) -> p o n", o=reps
                                ),
                                g16r,
                                op=mybir.AluOpType.min,
                            )
                        mv16 = m16[:].rearrange("p (s n) -> p s n", s=2)
                        dv16 = d16[:].rearrange("p (s n) -> p s n", s=2)
                        for c4 in range(NBLK // NCH):
                            c = nb * (NBLK // NCH) + c4
                            nsl = slice(c4 * NCH, (c4 + 1) * NCH)
                            for s in range(2):
                                acc_mm(c, sel2v[:, pair, c, s], mv16[:, s, nsl], False)
                                acc_mm(c, sel1v[:, pair, c, s], dv16[:, s, nsl], False)
                        continue
                    m_t = mpool.tile([128, 2 * NBLK], F8, tag="m")
                    nc.sync.dma_start(
                        out=m_t[:].rearrange("p (s n) -> p s n", s=2),
                        in_=memd[:][pair, :, :, nb * NBLK : (nb + 1) * NBLK],
                    )
                    mv = m_t[:].rearrange("p (s n) -> p s n", s=2)
                    # R2: raw-m sums (fire right after the DMA);
                    # skipped when the sel2 column is all-zero (both s abs)
                    for c4 in range(NBLK // NCH):
                        c = nb * (NBLK // NCH) + c4
                        if not _r2_needed(nb, pair, c):
                            continue
                        acc_mm(
                            c, sel2v[:, pair, c],
                            mv[:, :, c4 * NCH : (c4 + 1) * NCH], True,
                        )
                    # elementwise
                    diff = dpool.tile([128, 2 * NBLK], F8, tag="diff")
                    for s in range(2):
                        gc = g_cols[:, 2 * pair + s : 2 * pair + s + 1]
                        for h in range(2):
                            eng = SLICE_ENG[(nb, pair, s, h)]
                            rlo = h * (NBLK // 2)
                            rhi = rlo + NBLK // 2
                            lo = s * NBLK + rlo
                            hi = s * NBLK + rhi
                            if eng == "s":
                                nc.scalar.activation(
                                    diff[:, lo:hi],
                                    m_t[:, lo:hi],
                                    mybir.ActivationFunctionType.Abs,
                                    bias=gc,
                                    scale=-1.0,
                                )
                            else:
                                tt = 2 * pair + s
                                reps = (rhi - rlo) // 128
                                g8r = g8rep[
                                    :, tt * 128 : (tt + 1) * 128
                                ][:, None, :].broadcast_to([128, reps, 128])
                                nc.vector.tensor_tensor(
                                    diff[:, lo:hi].rearrange(
                                        "p (o n) -> p o n", o=reps
                                    ),
                                    m_t[:, lo:hi].rearrange(
                                        "p (o n) -> p o n", o=reps
                                    ),
                                    g8r,
                                    op=mybir.AluOpType.min,
                                )
                    # R1: signed sums of diff
                    dv = diff[:].rearrange("p (s n) -> p s n", s=2)
                    for c4 in range(NBLK // NCH):
                        c = nb * (NBLK // NCH) + c4
                        acc_mm(
                            c, sel1v[:, pair, c],
                            dv[:, :, c4 * NCH : (c4 + 1) * NCH], True,
                        )


            nc.vector.tensor_scalar(
                out_lo[:], acc_lo[:], g_col_sb[:, 0:1], None,
                op0=mybir.AluOpType.add,
            )
            nc.sync.dma_start(out=partials[:][0:16, :], in_=out_lo[0:16, :])
            out_hi = spool.tile([32, NCH], f16, tag="outhi")
            nc.vector.tensor_scalar(
                out_hi[:], acc_hi[:], g_col_sb[:, 0:1], None,
                op0=mybir.AluOpType.add,
            )
            nc.sync.dma_start(out=partials[:][16:32, :], in_=out_hi[16:32, :])

    nc.compile()
    return nc


def _get_nc():
    if "nc" not in _CACHE:
        _CACHE["nc"] = _build()
    return _CACHE["nc"]


def make_sel_arrays():
    """sel1/sel2 [128, pair, c, s, m] and selG [128, pair, s, m] fp8."""
    sel1 = np.zeros((128, PAIRS, 32, 2, 32), dtype=NP_F8)
    sel2 = np.zeros((128, PAIRS, 32, 2, 32), dtype=NP_F8)
    selg = np.zeros((128, PAIRS, 2, 32), dtype=NP_F8)
    cpb = NBLK // NCH  # chunks per block
    for pair in range(PAIRS):
        for c in range(32):
            nb, h = c // cpb, (c % cpb) // (cpb // 2)
            for s in range(2):
                if _slice_is_min(nb, pair, s, h):
                    sel1[:, pair, c, s, c] = -2.0
                    sel2[:, pair, c, s, c] = 1.0
                    selg[:, pair, s, c] = 1.0
                else:
                    sel1[:, pair, c, s, c] = 1.0
    return (
        sel1.reshape(128, -1),
        sel2.reshape(128, -1),
        selg.reshape(128, -1),
    )


def make_inputs(query, W, b, memory):
    """Host-side packing: transpose + fp8 cast + per-core layouts."""
    mem8t = np.ascontiguousarray(memory.astype(NP_F8).T)  # [D, N]
    mem16t = np.ascontiguousarray(memory.astype(np.float16).T)  # [D, N]
    W8 = (W * 8.0).astype(NP_F8)                          # [D, D] (rows j)
    q8 = (query.reshape(D) / 8.0).astype(NP_F8)
    q_dr = np.ascontiguousarray(
        q8.reshape(W_PAIRS, 2, 128).transpose(2, 1, 0)    # [p, s, t]
    ).reshape(128, 2 * W_PAIRS)
    sel1, sel2, selg = make_sel_arrays()
    one = np.ones((1, 128), dtype=np.float32)

    in_maps = []
    for c in range(N_CORES):
        base = c * D_SH
        # mem: [pair, p, s, n] with d = base + (2*pair+s)*128 + p
        marr = mem8t[base : base + D_SH]                  # [512, N] contig
        marr = np.ascontiguousarray(
            marr.reshape(PAIRS, 2, 128, N).transpose(0, 2, 1, 3)
        )
        m16full = mem16t[base : base + D_SH].reshape(PAIRS, 2, 128, N)
        if F16_BLOCKS:
            m16arr = np.stack(
                [
                    np.ascontiguousarray(
                        m16full[pair, :, :, nb * NBLK : (nb + 1) * NBLK]
                        .transpose(1, 0, 2)
                    )
                    for (nb, pair) in F16_BLOCKS
                ]
            )
        else:
            m16arr = np.zeros((1, 128, 2, NBLK), dtype=np.float16)
        # W: w_dr[p, t, s, l] = 8*W[base+l, t*256+s*128+p]
        wc = W8[base : base + D_SH, :]                    # [512 l, 4096 d]
        warr = np.ascontiguousarray(
            wc.T.reshape(W_PAIRS, 2, 128, D_SH).transpose(2, 0, 1, 3)
        ).reshape(128, W_PAIRS * 2 * D_SH)
        in_maps.append(
            {
                "qd": q_dr,
                "wd": warr,
                "bd": np.ascontiguousarray(
                    b[base : base + D_SH].astype(np.float32).reshape(4, 128).T
                ),
                "oned": one,
                "sel1d": sel1,
                "sel2d": sel2,
                "selgd": selg,
                "memd": marr,
                "memd16": m16arr,
            }
        )
    return in_maps


def kernel(query, W, b, memory, _trace=False, _return_raw=False):
    query = np.asarray(query, dtype=np.float32)
    W = np.asarray(W, dtype=np.float32)
    b = np.asarray(b, dtype=np.float32)
    memory = np.asarray(memory, dtype=np.float32)

    in_maps = make_inputs(query, W, b, memory)
    nc = _get_nc()
    res = run_bass_kernel_spmd(
        nc, in_maps, list(range(N_CORES)), trace=_trace
    )

    total = np.zeros(N, dtype=np.float64)
    for c in range(N_CORES):
        total += res.results[c]["partials"].astype(np.float64).reshape(N)
    sims = (1.0 - total / D).astype(np.float32)
    mask = sims >= THRESHOLD
    if _return_raw:
        return (sims, mask), res
    return sims, mask


# revision 4
# speedup vs baseline: 1.0024x; 1.0024x over previous
"""Trainium2 Bass kernel for nn_BinaryMemory (retrieval_knn) — v2.

reference:
    gated = sigmoid(query @ W.T + b)                      # [1, D], D=4096
    sims  = 1 - mean(|memory - gated|, axis=-1)           # [N],   N=16384
    mask  = sims >= 0.8

Sharding (8 cores, no collectives): shard the D axis; core c owns dims
[c*512, (c+1)*512).

v2 layout: d on PARTITIONS (transposed memory), fp8 everywhere.
  - gate: z = q @ W.T via fp8 DoubleRow matmuls (16 accumulating pairs),
    + b, sigmoid on ScalarE, then PE-transpose g into per-partition
    columns g_cols[128, 4] (one per d-subtile).
  - elementwise work split across THREE engines (the tensor_scalar ISA
    has no abs op, so DVE/GpSimd produce min while ScalarE produces abs):
      DVE:    diff = min(m, g)      tensor_scalar(op0=min)
      GpSimd: diff = min(m, g)      tensor_scalar(op0=min)
      Scalar: diff = |m - g|        activation(Abs, scale=-1, bias=g)
  - d-reduction on PE with SIGNED selector stationaries (fp8 DoubleRow):
    identity sum_d |m-g| = sum_abs |m-g| + sum_min (m + g - 2*min):
      R1: sel1 (col c: +1 abs-slices, -2 min-slices) over diff tiles
      R2: sel2 (col c: +1 min-slices only)           over RAW m tiles
      G:  selG (dense w2)                            over g8 columns
    R1+R2 accumulate into one psum [32, 512]; G into psum [32, 1];
    partial[c, n] = R1+R2 + G[c]  (added during evacuation).
  - evac: tensor_scalar_add(psum, G) -> sbuf f16 -> DMA [32, 512] out.

Per-core HBM: mem 8MB + W 2MB fp8 + sel ~1MB ~= 11.2MB.
"""
import sys

sys.path.insert(0, "/opt/trn_rl_repo")

import ml_dtypes
import numpy as np

import concourse.bacc as bacc
import concourse.mybir as mybir
import concourse.tile as tile
from concourse.bass_utils import run_bass_kernel_spmd

N_CORES = 8
D = 4096
N = 16384
D_SH = D // N_CORES          # 512 dims per core
PAIRS = 2                    # d-subtile DoubleRow pairs per core
W_PAIRS = D // 256           # 16 gate contraction pairs
NBLK = 4096                  # n per mem DMA block
NB = N // NBLK               # 8 blocks
NCH = 512                    # psum-bank n-chunk
THRESHOLD = 0.8

F8 = mybir.dt.float8e4
NP_F8 = ml_dtypes.float8_e4m3
F16_BLOCKS = ((2, 0),)   # (nb, pair) mem blocks shipped fp16 -> DVE 2x

_CACHE = {}


def _assign_slices():
    """Greedy-balance the 32 elementwise slices [128, 2048] across
    DVE(min) / Scalar(abs) / GpSimd(min) by modeled cost (ns/slice).
    Returns dict (nb, pair, s, half) -> 'v'|'s'|'g'."""
    cost = {"v": 2205.0, "s": 2001.0}
    load = {"v": len(F16_BLOCKS) * 2 * 2270.0 + 1400.0, "s": 0.0}
    out = {}
    for nb in range(NB):
        for pair in range(PAIRS):
            for s in range(2):
                for h in range(2):
                    if (nb, pair) in F16_BLOCKS:
                        out[(nb, pair, s, h)] = "v"
                        continue
                    e = min(cost, key=lambda k: load[k] + cost[k])
                    load[e] += cost[e]
                    out[(nb, pair, s, h)] = e
    return out


SLICE_ENG = _assign_slices()


def _r2_needed(nb, pair, c):
    cpb = NBLK // NCH
    h = (c % cpb) // (cpb // 2)
    return any(
        SLICE_ENG[(nb, pair, s, h)] != "s" for s in range(2)
    )


def _slice_is_min(nb, pair, s, h):
    return SLICE_ENG[(nb, pair, s, h)] != "s"


def _build():
    f32 = mybir.dt.float32
    f16 = mybir.dt.float16
    DR = mybir.MatmulPerfMode.DoubleRow
    nc = bacc.Bacc(
        "TRN2", target_bir_lowering=False, debug=False, num_devices=N_CORES
    )

    qd = nc.dram_tensor("qd", [128, 2 * W_PAIRS], F8, kind="ExternalInput")
    wd = nc.dram_tensor("wd", [128, W_PAIRS * 2 * D_SH], F8, kind="ExternalInput")
    bd = nc.dram_tensor("bd", [128, 4], f32, kind="ExternalInput")
    oned = nc.dram_tensor("oned", [1, 128], f32, kind="ExternalInput")
    sel1d = nc.dram_tensor("sel1d", [128, PAIRS * 32 * 2 * 32], F8, kind="ExternalInput")
    sel2d = nc.dram_tensor("sel2d", [128, PAIRS * 32 * 2 * 32], F8, kind="ExternalInput")
    selgd = nc.dram_tensor("selgd", [128, PAIRS * 2 * 32], F8, kind="ExternalInput")
    memd = nc.dram_tensor("memd", [PAIRS, 128, 2, N], F8, kind="ExternalInput")
    memd16 = nc.dram_tensor(
        "memd16", [max(1, len(F16_BLOCKS)), 128, 2, NBLK], f16,
        kind="ExternalInput",
    )
    partials = nc.dram_tensor("partials", [32, NCH], f16, kind="ExternalOutput")

    with tile.TileContext(nc) as tc:
        with (
            tc.tile_pool(name="const", bufs=1) as cpool,
            tc.tile_pool(name="mem", bufs=6) as mpool,
            tc.tile_pool(name="mem16", bufs=2) as m16pool,
            tc.tile_pool(name="diff", bufs=4) as dpool,
            tc.tile_pool(name="diff16", bufs=2) as d16pool,
            tc.tile_pool(name="small", bufs=1) as spool,
            tc.tile_pool(name="psum", bufs=1, space="PSUM") as ppool,
        ):
            # PE warmup: ramp the p-state before the gate matmuls arrive
            wrm = spool.tile([128, 1024], F8, tag="wrm")
            nc.vector.memset(wrm[:], 0.0)
            wrm_ps = ppool.tile([1, D_SH], f32, tag="wrmps")
            wv_warm = wrm[:].rearrange("p (s n) -> p s n", s=2)
            for i in range(10):
                nc.tensor.matmul(
                    wrm_ps[:],
                    wv_warm[:, :, 0:1],
                    wv_warm[:, :, 0:512],
                    start=(i == 0),
                    stop=(i == 9),
                    perf_mode=DR,
                )

            # scalar ring: q/b/one first (gate inputs), then selectors
            q_sb = cpool.tile([128, 2 * W_PAIRS], F8, tag="q")
            nc.scalar.dma_start(out=q_sb[:], in_=qd[:])
            bt_sb = spool.tile([128, 4], f32, tag="bt")
            nc.scalar.dma_start(out=bt_sb[:], in_=bd[:])
            one_sb = spool.tile([1, 128], f32, tag="one")
            nc.scalar.dma_start(out=one_sb[:], in_=oned[:])
            sel1_sb = cpool.tile([128, PAIRS * 32 * 2 * 32], F8, tag="sel1")
            nc.scalar.dma_start(out=sel1_sb[:], in_=sel1d[:])
            sel2_sb = cpool.tile([128, PAIRS * 32 * 2 * 32], F8, tag="sel2")
            nc.scalar.dma_start(out=sel2_sb[:], in_=sel2d[:])
            selg_sb = cpool.tile([128, PAIRS * 2 * 32], F8, tag="selg")
            nc.scalar.dma_start(out=selg_sb[:], in_=selgd[:])
            # bulk sync ring: W in 4 chunks (gate consumes per-chunk), then mem
            w_sb = cpool.tile([128, W_PAIRS * 2 * D_SH], F8, tag="w")
            WCH = W_PAIRS // 2
            for wc in range(2):
                lo = wc * WCH * 2 * D_SH
                hi = (wc + 1) * WCH * 2 * D_SH
                nc.sync.dma_start(out=w_sb[:, lo:hi], in_=wd[:][:, lo:hi])

            # ---- gate: z[l] = sum_d W[base+l, d] q[d] ----
            z_ps = ppool.tile([1, D_SH], f32, tag="zps")
            qv = q_sb[:].rearrange("p (s t) -> p s t", s=2)
            wv = w_sb[:].rearrange("p (t s l) -> p t s l", t=W_PAIRS, s=2)
            for t in range(W_PAIRS):
                nc.tensor.matmul(
                    z_ps[:],
                    qv[:, :, t : t + 1],
                    wv[:, t],
                    start=(t == 0),
                    stop=(t == W_PAIRS - 1),
                    perf_mode=DR,
                )
            # transpose z FIRST (tiny), then +b / sigmoid in column layout
            z_sb = spool.tile([1, D_SH], f16, tag="zsb")
            nc.vector.tensor_copy(z_sb[:], z_ps[:])
            one16 = spool.tile([1, 1], f16, tag="one16")
            nc.vector.tensor_copy(one16[:], one_sb[0:1, 0:1])
            zt_ps = ppool.tile([128, 4], f32, tag="ztps")
            for t in range(4):
                nc.tensor.matmul(
                    zt_ps[:, t : t + 1],
                    z_sb[0:1, t * 128 : (t + 1) * 128],
                    one16[:],
                    start=True,
                    stop=True,
                )
            zbt = spool.tile([128, 4], f32, tag="zbt")
            nc.vector.tensor_add(
                zbt[:], zt_ps[:], bt_sb[:]
            )
            g_cols = spool.tile([128, 4], f32, tag="gcols")
            nc.scalar.activation(
                g_cols[:], zbt[:], mybir.ActivationFunctionType.Sigmoid
            )
            g8rep = spool.tile([128, 4 * 128], F8, tag="g8rep")
            nc.vector.tensor_copy(
                g8rep[:].rearrange("p (t n) -> p t n", t=4),
                g_cols[:, :, None].broadcast_to([128, 4, 128]),
            )
            g8_cols = spool.tile([128, 4], F8, tag="g8cols")
            nc.vector.tensor_copy(g8_cols[:], g_cols[:])
            if F16_BLOCKS:
                g16rep = spool.tile([128, 4 * 128], f16, tag="g16rep")
                nc.vector.tensor_copy(
                    g16rep[:].rearrange("p (t n) -> p t n", t=4),
                    g_cols[:, :, None].broadcast_to([128, 4, 128]),
                )

            # ---- G[c] = sum_{min-slices} g  (per-chunk constant) ----
            gacc_ps = ppool.tile([32, 1], f32, tag="gacc")
            selgv = selg_sb[:].rearrange("p (a s m) -> p a s m", a=PAIRS, s=2)
            g8v = g8_cols[:].rearrange("p (a s) -> p a s", a=PAIRS)
            for pair in range(PAIRS):
                nc.tensor.matmul(
                    gacc_ps[:],
                    selgv[:, pair],
                    g8v[:, pair].rearrange("p (s o) -> p s o", o=1),
                    start=(pair == 0),
                    stop=(pair == PAIRS - 1),
                    perf_mode=DR,
                )
            g_col_sb = spool.tile([32, 1], f32, tag="gcol32")
            nc.vector.tensor_copy(g_col_sb[:], gacc_ps[:])

            # ---- mem stream: R2 (raw m), elementwise, R1 (diff) ----
            acc_lo = ppool.tile([32, NCH], f32, tag="acclo")
            acc_hi = ppool.tile([32, NCH], f32, tag="acchi")
            sel1v = sel1_sb[:].rearrange(
                "p (a c s m) -> p a c s m", a=PAIRS, c=32, s=2
            )
            sel2v = sel2_sb[:].rearrange(
                "p (a c s m) -> p a c s m", a=PAIRS, c=32, s=2
            )
            n_mm = {0: 0, 1: 0}
            _tot = {0: 0, 1: 0}
            for _nb in range(NB):
                for _pair in range(PAIRS):
                    for _c4 in range(NBLK // NCH):
                        _c = _nb * (NBLK // NCH) + _c4
                        _half = _c // 16
                        if (_nb, _pair) in F16_BLOCKS:
                            _tot[_half] += 4
                            continue
                        _tot[_half] += 1
                        if _r2_needed(_nb, _pair, _c):
                            _tot[_half] += 1
            n_mm_last = {h: _tot[h] - 1 for h in (0, 1)}

            def acc_mm(c, lhsT, rhs, dr):
                h = c // 16
                tile_ = acc_lo if h == 0 else acc_hi
                nc.tensor.matmul(
                    tile_[:],
                    lhsT,
                    rhs,
                    start=(n_mm[h] == 0),
                    stop=(n_mm[h] == n_mm_last[h]),
                    perf_mode=(
                        mybir.MatmulPerfMode.DoubleRow if dr else None
                    ),
                )
                n_mm[h] += 1
            f16_idx = {blk: i for i, blk in enumerate(F16_BLOCKS)}
            out_lo = spool.tile([32, NCH], f16, tag="outlo")
            for nb in range(NB):
                for pair in range(PAIRS):
                    if (nb, pair) in f16_idx:
                        bi = f16_idx[(nb, pair)]
                        m16 = m16pool.tile([128, 2 * NBLK], f16, tag="m16")
                        nc.sync.dma_start(
                            out=m16[:].rearrange("p (s n) -> p s n", s=2),
                            in_=memd16[:][bi],
                        )
                        d16 = d16pool.tile([128, 2 * NBLK], f16, tag="d16")
                        reps = NBLK // 128
                        for s in range(2):
                            tt = 2 * pair + s
                            g16r = g16rep[
                                :, tt * 128 : (tt + 1) * 128
                            ][:, None, :].broadcast_to([128, reps, 128])
                            lo = s * NBLK
                            hi = lo + NBLK
                            nc.vector.tensor_tensor(
                                d16[:, lo:hi].rearrange(
                                    "p (o n) -> p o n", o=reps
                                ),
                                m16[:, lo:hi].rearrange(
                                    "p (o n) -> p o n", o=reps
                                ),
                                g16r,
                                op=mybir.AluOpType.min,
                            )
                        mv16 = m16[:].rearrange("p (s n) -> p s n", s=2)
                        dv16 = d16[:].rearrange("p (s n) -> p s n", s=2)
                        for c4 in range(NBLK // NCH):
                            c = nb * (NBLK // NCH) + c4
                            nsl = slice(c4 * NCH, (c4 + 1) * NCH)
                            for s in range(2):
                                acc_mm(c, sel2v[:, pair, c, s], mv16[:, s, nsl], False)
                                acc_mm(c, sel1v[:, pair, c, s], dv16[:, s, nsl], False)
                        continue
                    m_t = mpool.tile([128, 2 * NBLK], F8, tag="m")
                    nc.sync.dma_start(
                        out=m_t[:].rearrange("p (s n) -> p s n", s=2),
                        in_=memd[:][pair, :, :, nb * NBLK : (nb + 1) * NBLK],
                    )
                    mv = m_t[:].rearrange("p (s n) -> p s n", s=2)
                    # R2: raw-m sums (fire right after the DMA);
                    # skipped when the sel2 column is all-zero (both s abs)
                    for c4 in range(NBLK // NCH):
                        c = nb * (NBLK // NCH) + c4
                        if not _r2_needed(nb, pair, c):
                            continue
                        acc_mm(
                            c, sel2v[:, pair, c],
                            mv[:, :, c4 * NCH : (c4 + 1) * NCH], True,
                        )
                    # elementwise
                    diff = dpool.tile([128, 2 * NBLK], F8, tag="diff")
                    for s in range(2):
                        gc = g_cols[:, 2 * pair + s : 2 * pair + s + 1]
                        for h in range(2):
                            eng = SLICE_ENG[(nb, pair, s, h)]
                            rlo = h * (NBLK // 2)
                            rhi = rlo + NBLK // 2
                            lo = s * NBLK + rlo
                            hi = s * NBLK + rhi
                            if eng == "s":
                                nc.scalar.activation(
                                    diff[:, lo:hi],
                                    m_t[:, lo:hi],
                                    mybir.ActivationFunctionType.Abs,
                                    bias=gc,
                                    scale=-1.0,
                                )
                            else:
                                tt = 2 * pair + s
                                reps = (rhi - rlo) // 128
                                g8r = g8rep[
                                    :, tt * 128 : (tt + 1) * 128
                                ][:, None, :].broadcast_to([128, reps, 128])
                                nc.vector.tensor_tensor(
                                    diff[:, lo:hi].rearrange(
                                        "p (o n) -> p o n", o=reps
                                    ),
                                    m_t[:, lo:hi].rearrange(
                                        "p (o n) -> p o n", o=reps
                                    ),
                                    g8r,
                                    op=mybir.AluOpType.min,
                                )
                    # R1: signed sums of diff
                    dv = diff[:].rearrange("p (s n) -> p s n", s=2)
                    for c4 in range(NBLK // NCH):
                        c = nb * (NBLK // NCH) + c4
                        acc_mm(
                            c, sel1v[:, pair, c],
                            dv[:, :, c4 * NCH : (c4 + 1) * NCH], True,
                        )


            nc.vector.tensor_scalar(
                out_lo[:], acc_lo[:], g_col_sb[:, 0:1], None,
                op0=mybir.AluOpType.add,
            )
            nc.sync.dma_start(out=partials[:][0:16, :], in_=out_lo[0:16, :])
            out_hi = spool.tile([32, NCH], f16, tag="outhi")
            nc.vector.tensor_scalar(
                out_hi[:], acc_hi[:], g_col_sb[:, 0:1], None,
                op0=mybir.AluOpType.add,
            )
            nc.sync.dma_start(out=partials[:][16:32, :], in_=out_hi[16:32, :])

    nc.compile()
    return nc


def _get_nc():
    if "nc" not in _CACHE:
        _CACHE["nc"] = _build()
    return _CACHE["nc"]


def make_sel_arrays():
    """sel1/sel2 [128, pair, c, s, m] and selG [128, pair, s, m] fp8."""
    sel1 = np.zeros((128, PAIRS, 32, 2, 32), dtype=NP_F8)
    sel2 = np.zeros((128, PAIRS, 32, 2, 32), dtype=NP_F8)
    selg = np.zeros((128, PAIRS, 2, 32), dtype=NP_F8)
    cpb = NBLK // NCH  # chunks per block
    for pair in range(PAIRS):
        for c in range(32):
            nb, h = c // cpb, (c % cpb) // (cpb // 2)
            for s in range(2):
                if _slice_is_min(nb, pair, s, h):
                    sel1[:, pair, c, s, c] = -2.0
                    sel2[:, pair, c, s, c] = 1.0
                    selg[:, pair, s, c] = 1.0
                else:
                    sel1[:, pair, c, s, c] = 1.0
    return (
        sel1.reshape(128, -1),
        sel2.reshape(128, -1),
        selg.reshape(128, -1),
    )


def make_inputs(query, W, b, memory):
    """Host-side packing: transpose + fp8 cast + per-core layouts."""
    mem8t = np.ascontiguousarray(memory.astype(NP_F8).T)  # [D, N]
    mem16t = np.ascontiguousarray(memory.astype(np.float16).T)  # [D, N]
    W8 = (W * 8.0).astype(NP_F8)                          # [D, D] (rows j)
    q8 = (query.reshape(D) / 8.0).astype(NP_F8)
    q_dr = np.ascontiguousarray(
        q8.reshape(W_PAIRS, 2, 128).transpose(2, 1, 0)    # [p, s, t]
    ).reshape(128, 2 * W_PAIRS)
    sel1, sel2, selg = make_sel_arrays()
    one = np.ones((1, 128), dtype=np.float32)

    in_maps = []
    for c in range(N_CORES):
        base = c * D_SH
        # mem: [pair, p, s, n] with d = base + (2*pair+s)*128 + p
        marr = mem8t[base : base + D_SH]                  # [512, N] contig
        marr = np.ascontiguousarray(
            marr.reshape(PAIRS, 2, 128, N).transpose(0, 2, 1, 3)
        )
        m16full = mem16t[base : base + D_SH].reshape(PAIRS, 2, 128, N)
        if F16_BLOCKS:
            m16arr = np.stack(
                [
                    np.ascontiguousarray(
                        m16full[pair, :, :, nb * NBLK : (nb + 1) * NBLK]
                        .transpose(1, 0, 2)
                    )
                    for (nb, pair) in F16_BLOCKS
                ]
            )
        else:
            m16arr = np.zeros((1, 128, 2, NBLK), dtype=np.float16)
        # W: w_dr[p, t, s, l] = 8*W[base+l, t*256+s*128+p]
        wc = W8[base : base + D_SH, :]                    # [512 l, 4096 d]
        warr = np.ascontiguousarray(
            wc.T.reshape(W_PAIRS, 2, 128, D_SH).transpose(2, 0, 1, 3)
        ).reshape(128, W_PAIRS * 2 * D_SH)
        in_maps.append(
            {
                "qd": q_dr,
                "wd": warr,
                "bd": np.ascontiguousarray(
                    b[base : base + D_SH].astype(np.float32).reshape(4, 128).T
                ),
                "oned": one,
                "sel1d": sel1,
                "sel2d": sel2,
                "selgd": selg,
                "memd": marr,
                "memd16": m16arr,
            }
        )
    return in_maps


def kernel(query, W, b, memory, _trace=False, _return_raw=False):
    query = np.asarray(query, dtype=np.float32)
    W = np.asarray(W, dtype=np.float32)
    b = np.asarray(b, dtype=np.float32)
    memory = np.asarray(memory, dtype=np.float32)

    in_maps = make_inputs(query, W, b, memory)
    nc = _get_nc()
    res = run_bass_kernel_spmd(
        nc, in_maps, list(range(N_CORES)), trace=_trace
    )

    total = np.zeros(N, dtype=np.float64)
    for c in range(N_CORES):
        total += res.results[c]["partials"].astype(np.float64).reshape(N)
    sims = (1.0 - total / D).astype(np.float32)
    mask = sims >= THRESHOLD
    if _return_raw:
        return (sims, mask), res
    return sims, mask
